# revision 4
# baseline (speedup 1.0000x reference)
"""ECPGLinear (ternary-quantized linear) Bass kernel for 8 TRN2 NeuronCores.

Computes out = x @ W.T where W = dequant(ternary, per-group scales),
group_size=128 along in_features.

Sharding: data-parallel over the 8192 (batch*seq) tokens — each core takes
1024 rows of x and the full weight matrix; no collectives, the host
concatenates the 8 output shards.

Per-core schedule (dequant + matmul on-device, fp16 compute):
  - X^T shard resident in SBUF (cast f32 -> fp16 during the load DMA).
  - Weight-tile production (scb DMA on sync + ternary DMA on gpsimd ->
    DVE dequant-multiply) runs P tiles AHEAD of matmul consumption in
    program order, so the dequant pipeline is already full when the
    first real matmul issues and chunk-boundary eviction blocks never
    delay the next chunk's first tiles.
  - Per consumed tile: 8 matmuls (one per 128-row m-tile) accumulate
    into 8 PSUM banks over the 32 k-tiles of each 512-col output chunk.
  - ACT evicts PSUM to SBUF as fp16 (host casts back to f32; ~2e-4 rel
    err vs the 2e-2 budget); stores go out via the gpsimd SWDGE queue.

Tail scheduling (trace-driven): the last chunk is split into two 4-bank
phases (m 0-3 then m 4-7) with the dequantized W tiles kept resident and
reused, so phase A's evictions/stores overlap phase B's matmuls and only
4 copies + 4 stores drain after the final matmul, split across DVE/ACT
and the sync/scalar HWDGE queues. Warmup is 8 matmuls: the PE-clock ramp
(~4.5us at reduced rate) covers the first tiles' DMA+sem latency.

Host prep is layout-only: transpose/shard/dtype-cast and replication of
the per-group scales across the 128 partitions. Since ternary is in
{-1,0,1}, rounding scales to fp16 on the host is bit-identical to
dequantizing in fp32 on-device and rounding: fp16(t*s) == t*fp16(s).
"""
import functools
import numpy as np

OUT_F = 4096
IN_F = 4096
B, S = 4, 2048
M_TOT = B * S             # 8192 tokens
NCORES = 8
M_CORE = M_TOT // NCORES  # 1024 tokens per core
KT = IN_F // 128          # 32 contraction tiles
NCH = OUT_F // 512        # 8 output chunks of 512
MT = M_CORE // 128        # 8 m-tiles per core
NWARM = 8
PREF = 5                  # production runs PREF tiles ahead of consumption
TOT = NCH * KT            # 256 weight tiles


@functools.lru_cache(maxsize=1)
def _build():
    from concourse import bacc
    import concourse.mybir as mybir
    import concourse.tile as tile

    f32 = mybir.dt.float32
    f16 = mybir.dt.float16

    nc = bacc.Bacc("TRN2", target_bir_lowering=False, debug=False,
                   num_devices=NCORES)
    xt = nc.dram_tensor("xt", [IN_F, M_CORE], f16, kind="ExternalInput")
    tt = nc.dram_tensor("tt", [IN_F, OUT_F], mybir.dt.int8, kind="ExternalInput")
    # scales pre-replicated across partitions: [KT, 128, OUT_F]
    sc = nc.dram_tensor("sc", [KT, 128, OUT_F], f16, kind="ExternalInput")

    out = nc.dram_tensor("out", [M_CORE, OUT_F], f16, kind="ExternalOutput")

    with tile.TileContext(nc) as tc:
        with (
            tc.tile_pool(name="xres", bufs=1) as xres_pool,
            tc.tile_pool(name="scb", bufs=8) as scb_pool,
            tc.tile_pool(name="tern", bufs=8) as tern_pool,
            tc.tile_pool(name="wd", bufs=8) as wd_pool,
            tc.tile_pool(name="wdl", bufs=KT) as wdl_pool,
            tc.tile_pool(name="ost", bufs=12) as ost_pool,
            tc.tile_pool(name="psum", bufs=8, space="PSUM") as psum_pool,
        ):
            # Resident X^T: [128 part, KT, M_CORE]
            xres = xres_pool.tile([128, KT, M_CORE], f16)

            # PE warmup: keep the HAM busy while the first tiles load.
            warm_l = scb_pool.tile([128, 128], f16, name="warm_l",
                                   tag="warm")
            warm_r = tern_pool.tile([128, 512], f16, name="warm_r",
                                    tag="warm_r")
            nc.vector.memset(warm_l[:], 0.0)
            nc.vector.memset(warm_r[:], 0.0)
            warm_ps = psum_pool.tile([128, 512], f32, name="warm_ps",
                                     tag="ps")
            for _ in range(NWARM):
                nc.tensor.matmul(warm_ps[:], warm_l[:], warm_r[:],
                                 start=True, stop=True)

            wds = {}

            def produce(j):
                """DMA scb+tern for tile j and DVE-dequantize."""
                n, kt = divmod(j, KT)
                o0 = n * 512
                scb = scb_pool.tile([128, 512], f16,
                                    name=f"scb{n}_{kt}", tag="scb")
                nc.sync.dma_start(scb[:], sc[kt, :, o0:o0 + 512])
                if n == 0:
                    nc.sync.dma_start(xres[:, kt, :],
                                      xt[kt * 128:(kt + 1) * 128, :])
                tern = tern_pool.tile([128, 512], mybir.dt.int8,
                                      name=f"tern{n}_{kt}", tag="tern")
                nc.gpsimd.dma_start(
                    tern[:], tt[kt * 128:(kt + 1) * 128, o0:o0 + 512])
                pool = wdl_pool if n == NCH - 1 else wd_pool
                wd = pool.tile([128, 512], f16, name=f"wd{n}_{kt}",
                               tag="wdl" if pool is wdl_pool else "wd")
                nc.vector.tensor_mul(wd[:], tern[:], scb[:])
                wds[j] = wd

            psums = None

            def consume(j):
                nonlocal psums
                n, kt = divmod(j, KT)
                last = n == NCH - 1
                ms = range(0, 4) if last else range(MT)
                if kt == 0:
                    psums = [psum_pool.tile([128, 512], f32,
                                            name=f"ps{n}_{m}", tag="ps")
                             for m in ms]
                wd = wds.pop(j) if not last else wds[j]
                for i, m in enumerate(ms):
                    nc.tensor.matmul(
                        psums[i][:],
                        xres[:, kt, m * 128:(m + 1) * 128],
                        wd[:],
                        start=(kt == 0),
                        stop=(kt == KT - 1),
                    )
                if kt == KT - 1:
                    o0 = n * 512
                    if not last:
                        for i, m in enumerate(ms):
                            ost = ost_pool.tile([128, 512], f16,
                                                name=f"ost{n}_{m}",
                                                tag="ost")
                            nc.scalar.copy(ost[:], psums[i][:])
                            nc.gpsimd.dma_start(
                                out[m * 128:(m + 1) * 128, o0:o0 + 512],
                                ost[:])

            for j in range(TOT + PREF):
                if j < TOT:
                    produce(j)
                jc = j - PREF
                if jc >= 0:
                    consume(jc)

            # Last chunk finale: evict phase A (m 0-3, overlapped with
            # phase B), run phase B (m 4-7) on the resident wdl tiles,
            # then drain only 4 copies + 4 stores split across DVE/ACT
            # and the sync/scalar HWDGE queues.
            n = NCH - 1
            o0 = n * 512
            psA = psums
            for half, ms in ((0, range(0, 4)), (1, range(4, MT))):
                if half == 1:
                    psB = [psum_pool.tile([128, 512], f32,
                                          name=f"ps{n}_{m}", tag="ps")
                           for m in ms]
                    for kt in range(KT):
                        wd = wds[n * KT + kt]
                        for i, m in enumerate(ms):
                            nc.tensor.matmul(
                                psB[i][:],
                                xres[:, kt, m * 128:(m + 1) * 128],
                                wd[:],
                                start=(kt == 0),
                                stop=(kt == KT - 1),
                            )
                ps = psA if half == 0 else psB
                for i, m in enumerate(ms):
                    ost = ost_pool.tile([128, 512], f16,
                                        name=f"ost{n}_{m}", tag="ost")
                    if m % 2 == 0:
                        nc.vector.tensor_copy(ost[:], ps[i][:])
                    else:
                        nc.scalar.copy(ost[:], ps[i][:])
                    if half == 0:
                        dma = nc.gpsimd
                    else:
                        dma = nc.sync if m % 2 == 0 else nc.scalar
                    dma.dma_start(
                        out[m * 128:(m + 1) * 128, o0:o0 + 512], ost[:])

    nc.compile()
    return nc


def kernel(x: np.ndarray, ternary: np.ndarray, scales: np.ndarray,
           _trace: bool = False):
    from concourse.bass_utils import run_bass_kernel_spmd

    nc = _build()

    x = np.asarray(x)
    ternary = np.asarray(ternary)
    scales = np.asarray(scales)

    xf = x.reshape(M_TOT, IN_F)
    ttm = np.ascontiguousarray(ternary.T.astype(np.int8))
    # scales as [KT, OUT_F] (sc[kt, o] = scales[o*KT + kt]), replicated
    # across the 128 partitions: [KT, 128, OUT_F]
    scm = np.ascontiguousarray(scales.reshape(OUT_F, KT).T.astype(np.float16))
    scr = np.ascontiguousarray(
        np.broadcast_to(scm[:, None, :], (KT, 128, OUT_F)))

    in_maps = []
    for c in range(NCORES):
        xc = np.ascontiguousarray(
            xf[c * M_CORE:(c + 1) * M_CORE, :].T.astype(np.float16))
        in_maps.append({"xt": xc, "tt": ttm, "sc": scr})

    res = run_bass_kernel_spmd(nc, in_maps, list(range(NCORES)),
                               trace=_trace)
    outs = [res.results[c]["out"] for c in range(NCORES)]
    full = np.concatenate(outs, axis=0).astype(np.float32).reshape(B, S, OUT_F)
    if _trace:
        kernel.last_results = res
    return full


kernel.last_results = None
